# revision 1
# baseline (speedup 1.0000x reference)
"""CodeWiseAttention kernel for Trainium2 (8 NeuronCores, label-dim sharded).

m[b,n,:] = softmax(label_feature[n] @ x[b].T) @ x[b]

Sharding: label rows N=8922 split across 8 cores (1116/core, padded to 1152);
x replicated. Per core, per batch:
  mm1 (fp32r): S^T[l,n] = xT[e,l].T @ labelT[e,n]     (xT via PE transpose)
  exp on ScalarE: expS = exp(S - 30)                   (constant shift; cancels)
  mm2 (fp32r): Uaug^T[e',n] += xa[l,e'].T @ expS^T[l,n]  accumulated over l,
      where xa has a ones column so row 100 of Uaug = Z = sum_l expS.
  out: PE-transpose Uaug^T -> [n, e'], m = U / Z, DMA out.
"""
import numpy as np
from contextlib import ExitStack

import concourse.tile as tile
from concourse import bacc, mybir
from concourse.bass_utils import run_bass_kernel_spmd

F32 = mybir.dt.float32
F32R = mybir.dt.float32r

B, L, E = 8, 2500, 100
LP = 2520          # L padded with zero rows (zero rows add nothing to U or Z)
N_TOTAL = 8922
NCORES = 8
NS = 1116          # label rows per core (core 7: 1110 real)
NSP = 1152         # padded per-core label rows
LC = 126           # l-chunk rows (even: fp32r ISA needs even innermost counts)
NLC = LP // LC     # 20 l-chunks
NCH = 384          # n-chunk width (moving dim; >=256 keeps fp32r at full rate)
NJ = NSP // NCH    # 3 n-chunks
EA = E + 1         # x augmented with ones column
PSB = 512          # psum bank stride in f32 elements
EXP_BIAS = -30.0

TRACE = False
LAST_RESULT = None

_NC = []


def _build():
    nc = bacc.Bacc("TRN2", target_bir_lowering=False, debug=False)
    xa_d = nc.dram_tensor("xa", [B, LP, EA], F32R, kind="ExternalInput").ap()
    lab_d = nc.dram_tensor("lab", [NSP, E], F32R, kind="ExternalInput").ap()
    idr_d = nc.dram_tensor("idr", [128, 128], F32R, kind="ExternalInput").ap()
    idf_d = nc.dram_tensor("idf", [128, 128], F32, kind="ExternalInput").ap()
    m_d = nc.dram_tensor("m", [B, NSP, E], F32, kind="ExternalOutput").ap()

    with tile.TileContext(nc) as tc, ExitStack() as ctx:
        consts = ctx.enter_context(tc.tile_pool(name="consts", bufs=1))
        lab_pool = ctx.enter_context(tc.tile_pool(name="labp", bufs=2))
        xa_pool = ctx.enter_context(tc.tile_pool(name="xap", bufs=2))
        xt_pool = ctx.enter_context(tc.tile_pool(name="xtp", bufs=2))
        e_pool = ctx.enter_context(tc.tile_pool(name="ep", bufs=3))
        u_pool = ctx.enter_context(tc.tile_pool(name="up", bufs=3))
        o_pool = ctx.enter_context(tc.tile_pool(name="op", bufs=4))
        r_pool = ctx.enter_context(tc.tile_pool(name="rp", bufs=4))
        pstr = ctx.enter_context(tc.tile_pool(name="pstr", bufs=2, space="PSUM"))
        pss = ctx.enter_context(tc.tile_pool(name="pss", bufs=1, space="PSUM"))
        psm = ctx.enter_context(tc.tile_pool(name="psm", bufs=1, space="PSUM"))

        idr_sb = consts.tile([128, 128], F32R)
        nc.sync.dma_start(out=idr_sb[:], in_=idr_d)
        idf_sb = consts.tile([128, 128], F32)
        nc.sync.dma_start(out=idf_sb[:], in_=idf_d)
        bias_sb = consts.tile([128, 1], F32)
        nc.vector.memset(bias_sb[:], EXP_BIAS)

        # labelT [E, NSP] via PE transposes of 128-row label chunks
        labT = consts.tile([E, NSP], F32R)
        for k in range(NSP // 128):
            lsb = lab_pool.tile([128, E], F32R, tag="lab")
            nc.sync.dma_start(out=lsb[:], in_=lab_d[k * 128:(k + 1) * 128, :])
            tp = pstr.tile([128, 128], F32R, tag="tr")
            nc.tensor.transpose(tp[:E, :], lsb[:], idr_sb[:, :])
            nc.vector.tensor_copy(labT[:, k * 128:(k + 1) * 128], tp[:E, :])

        # prologue DMA for batch 0; per-batch DMA for b+1 is issued before
        # batch b's compute so the transfer hides under the c-loop
        xa_tiles = {}
        xa_tiles[0] = xa_pool.tile([LC, NLC, EA], F32R, tag="xa", name="xa_sb0")
        nc.sync.dma_start(
            out=xa_tiles[0][:], in_=xa_d[0].rearrange("(c p) e -> p c e", p=LC)
        )
        for b in range(B):
            xa_sb = xa_tiles.pop(b)
            if b + 1 < B:
                xa_tiles[b + 1] = xa_pool.tile(
                    [LC, NLC, EA], F32R, tag="xa", name=f"xa_sb{b+1}")
                nc.sync.dma_start(
                    out=xa_tiles[b + 1][:],
                    in_=xa_d[b + 1].rearrange("(c p) e -> p c e", p=LC),
                )
            # xT [E, LP] for this batch
            xT = xt_pool.tile([E, LP], F32R, tag="xt")
            for c in range(NLC):
                tp = pstr.tile([128, 128], F32R, tag="tr")
                nc.tensor.transpose(
                    tp[:E, :LC], xa_sb[:, c, 0:E], idr_sb[:LC, :LC]
                )
                nc.vector.tensor_copy(xT[:, c * LC:(c + 1) * LC], tp[:E, :LC])

            # two passes over l-chunks: j in {0,1}, then j=2. Halving the
            # S^T tile lets it double-buffer inside 8 PSUM banks, so
            # mm1(c+1) never waits on exp(c).
            u_sbs = []
            for jlo, jn in ((0, 2), (2, 1)):
                m_ps = psm.tile([EA, 2, PSB], F32, tag="m")
                for c in range(NLC):
                    s_ps = pss.tile([LC, 2, PSB], F32, tag="s")
                    for jj in range(jn):
                        nc.tensor.matmul(
                            s_ps[:, jj, 0:NCH],
                            xT[:, c * LC:(c + 1) * LC],
                            labT[:, (jlo + jj) * NCH:(jlo + jj + 1) * NCH],
                        )
                    e_sb = e_pool.tile([LC, 2, NCH], F32R, tag="e")
                    nc.scalar.activation(
                        e_sb[:, 0:jn, :], s_ps[:, 0:jn, 0:NCH],
                        mybir.ActivationFunctionType.Exp,
                        bias=bias_sb[:LC], scale=1.0,
                    )
                    for jj in range(jn):
                        nc.tensor.matmul(
                            m_ps[:, jj, 0:NCH],
                            xa_sb[:, c, :],
                            e_sb[:, jj, :],
                            start=(c == 0), stop=(c == NLC - 1),
                        )
                u_sb = u_pool.tile([EA, 2, NCH], F32, tag="u")
                nc.vector.tensor_copy(
                    u_sb[:, 0:jn, :], m_ps[:, 0:jn, 0:NCH]
                )
                u_sbs.append(u_sb)

            # out path: U^T -> transpose -> divide by Z -> DMA
            for k in range(NSP // 128):
                j, off = divmod(k * 128, NCH)
                u_src = u_sbs[0][:, j, off:off + 128] if j < 2 else \
                    u_sbs[1][:, 0, off:off + 128]
                tpo = pstr.tile([128, 128], F32, tag="tr")
                nc.tensor.transpose(
                    tpo[:, :EA], u_src, idf_sb[:EA, :EA]
                )
                rz = r_pool.tile([128, 1], F32, tag="r")
                nc.vector.reciprocal(rz[:], tpo[:, E:EA])
                o_sb = o_pool.tile([128, E], F32, tag="o")
                nc.vector.tensor_scalar_mul(o_sb[:], tpo[:, 0:E], rz[:])
                nc.sync.dma_start(
                    out=m_d[b, k * 128:(k + 1) * 128, :], in_=o_sb[:]
                )
    nc.compile()
    return nc


def _get_nc():
    if not _NC:
        _NC.append(_build())
    return _NC[0]


def kernel(x, label_feature):
    global LAST_RESULT
    x = np.ascontiguousarray(np.asarray(x, dtype=np.float32))
    lf = np.ascontiguousarray(np.asarray(label_feature, dtype=np.float32))
    assert x.shape == (B, L, E) and lf.shape == (N_TOTAL, E)

    xa = np.zeros((B, LP, EA), np.float32)
    xa[:, :L, :E] = x
    xa[:, :L, E] = 1.0
    ident = np.eye(128, dtype=np.float32)
    in_maps = []
    for r in range(NCORES):
        lo = r * NS
        hi = min(lo + NS, N_TOTAL)
        shard = np.zeros((NSP, E), np.float32)
        shard[: hi - lo] = lf[lo:hi]
        in_maps.append({"xa": xa, "lab": shard, "idr": ident, "idf": ident})

    nc = _get_nc()
    res = run_bass_kernel_spmd(
        nc, in_maps, core_ids=list(range(NCORES)), trace=TRACE
    )
    LAST_RESULT = res

    out = np.empty((B, N_TOTAL, E), np.float32)
    for r in range(NCORES):
        lo = r * NS
        hi = min(lo + NS, N_TOTAL)
        out[:, lo:hi, :] = res.results[r]["m"][:, : hi - lo, :]
    return out



# revision 3
# speedup vs baseline: 3.0441x; 3.0441x over previous
"""CodeWiseAttention kernel for Trainium2 (8 NeuronCores, label-dim sharded).

m[b,n,:] = softmax(label_feature[n] @ x[b].T) @ x[b]

Sharding: label rows N=8922 split across 8 cores (1116/core, padded to 1152);
x replicated.  Per core, per batch b:
  mm1 (fp16):  S^T[l,n] = xT[e,l].T @ labT[e,n]     (xT, labT pre-transposed
               on host; fp16 runs the PE at 1 cycle/row vs ~3.5 for fp32)
  exp on ScalarE: expS = exp(S - 30) -> bf16        (constant shift; cancels)
  mm2 (bf16):  Uaug^T[e',n] += xa[l,e'].T @ expS^T[l,n]  accumulated over l,
               where xa has a ones column so row 100 of Uaug = Z = sum_l expS.
  out: DMA Uaug^T [101, n] to DRAM; host divides by Z and transposes.

PSUM layout (8 banks x 512 fp32 per partition), chosen so every matmul
output region sits inside one bank, accumulator banks are never touched by
score writes (start=True clears has_written for the WHOLE bank), and no
bank is PE-written while ScalarE/VectorE reads it (fatal collision; the
Tile tracker would serialize):
  b0 [0:512]      scores buf A, n[0:512]     } one contiguous 1024-wide
  b1 [512:1024]   scores buf A, n[512:1024]  }   ACTIVATE per l-chunk
  b2 [1024:1536]  scores buf B, n[0:512]
  b3 [1536:2048]  scores buf B, n[512:1024]
  b4 [2048:2560]  U accum, n[0:512]
  b5 [2560:3072]  U accum, n[512:1024]
  b6 [3072:3584]  "pack" scores: n[1024:1152] for 4 l-chunks at once
  b7 [3584:3712]  U accum, n[1024:1152]
The 128-wide n[1024:1152] tail is handled in 5 packs of 4 l-chunks so its
exp also runs as a few large ACTIVATEs (~293ns fixed cost per ACTIVATE).
"""
import numpy as np
import ml_dtypes
from contextlib import ExitStack

import concourse.tile as tile
from concourse import bacc, mybir
from concourse.bass_utils import run_bass_kernel_spmd

F32 = mybir.dt.float32
F16 = mybir.dt.float16
BF16 = mybir.dt.bfloat16

B, L, E = 8, 2500, 100
LP = 2520          # L padded; pad rows have xa=0 (incl. ones col) so they
                   # contribute nothing to U or Z even though exp(0-30) != 0
N_TOTAL = 8922
NCORES = 8
NS = 1116          # label rows per core (core 7: 1110 real)
NSP = 1152         # padded per-core label rows
NMAIN = 1024       # n columns handled by the main (512,512) loop
NT = 128           # tail n columns handled by 4-packs
LC = 126           # l-chunk rows (partition dim of S^T)
NLC = LP // LC     # 20 l-chunks
PACK = 4           # l-chunks per tail pack
NPACK = NLC // PACK
EA = E + 1         # x augmented with ones column
EXP_BIAS = -30.0

SA0, SB0 = 0, 1024     # main score buffer offsets (f32 elems)
U0 = 2048              # main U accumulator offset
PS0 = 3072             # pack score offset
UT0 = 3584             # tail U accumulator offset

TRACE = False
LAST_RESULT = None

_NC = []


def _build():
    nc = bacc.Bacc("TRN2", target_bir_lowering=False, debug=False)
    xt_d = nc.dram_tensor("xt", [B, E, LP], F16, kind="ExternalInput").ap()
    xa_d = nc.dram_tensor("xa", [B, LP, EA], BF16, kind="ExternalInput").ap()
    lab_d = nc.dram_tensor("labT", [E, NSP], F16, kind="ExternalInput").ap()
    m_d = nc.dram_tensor("m", [B, EA, NSP], F32, kind="ExternalOutput").ap()

    with tile.TileContext(nc) as tc, ExitStack() as ctx:
        consts = ctx.enter_context(tc.tile_pool(name="consts", bufs=1))
        xt_pool = ctx.enter_context(tc.tile_pool(name="xtp", bufs=2))
        xa_pool = ctx.enter_context(tc.tile_pool(name="xap", bufs=2))
        e_pool = ctx.enter_context(tc.tile_pool(name="ep", bufs=3))
        et_pool = ctx.enter_context(tc.tile_pool(name="etp", bufs=2))
        u_pool = ctx.enter_context(tc.tile_pool(name="up", bufs=2))
        ps = ctx.enter_context(tc.tile_pool(name="ps", bufs=1, space="PSUM"))

        arena = ps.tile([128, 4096], F32)

        labT = consts.tile([E, NSP], F16)
        nc.sync.dma_start(out=labT[:], in_=lab_d)
        bias_sb = consts.tile([128, 1], F32)
        nc.vector.memset(bias_sb[:], EXP_BIAS)

        xt_tiles, xa_tiles = {}, {}

        def fetch(b):
            xt_tiles[b] = xt_pool.tile([E, LP], F16, tag="xt", name=f"xt{b}")
            nc.sync.dma_start(out=xt_tiles[b][:], in_=xt_d[b])
            xa_tiles[b] = xa_pool.tile(
                [LC, NLC, EA], BF16, tag="xa", name=f"xa{b}")
            nc.sync.dma_start(
                out=xa_tiles[b][:],
                in_=xa_d[b].rearrange("(c p) e -> p c e", p=LC),
            )

        fetch(0)
        for b in range(B):
            xT = xt_tiles.pop(b)
            xa_sb = xa_tiles.pop(b)
            if b + 1 < B:
                fetch(b + 1)

            e_sbs, pe_sbs = {}, {}

            def mm1(c):
                base = SA0 if c % 2 == 0 else SB0
                for j in range(2):
                    nc.tensor.matmul(
                        arena[:LC, base + j * 512:base + (j + 1) * 512],
                        xT[:, c * LC:(c + 1) * LC],
                        labT[:, j * 512:(j + 1) * 512],
                    )

            def act(c):
                base = SA0 if c % 2 == 0 else SB0
                e_sb = e_pool.tile([128, NMAIN], BF16, tag="e", name=f"e{c}")
                nc.scalar.activation(
                    e_sb[:LC, :], arena[:LC, base:base + NMAIN],
                    mybir.ActivationFunctionType.Exp,
                    bias=bias_sb[:LC], scale=1.0,
                )
                e_sbs[c] = e_sb

            def mm2(c):
                e_sb = e_sbs.pop(c)
                for j in range(2):
                    nc.tensor.matmul(
                        arena[:EA, U0 + j * 512:U0 + (j + 1) * 512],
                        xa_sb[:, c, :],
                        e_sb[:LC, j * 512:(j + 1) * 512],
                        start=(c == 0), stop=(c == NLC - 1),
                    )

            def pack_mm1(p):
                for i in range(PACK):
                    c = p * PACK + i
                    nc.tensor.matmul(
                        arena[:LC, PS0 + i * NT:PS0 + (i + 1) * NT],
                        xT[:, c * LC:(c + 1) * LC],
                        labT[:, NMAIN:NSP],
                    )

            def pack_act(p):
                pe = et_pool.tile([128, PACK * NT], BF16, tag="pe",
                                  name=f"pe{p}")
                nc.scalar.activation(
                    pe[:LC, :], arena[:LC, PS0:PS0 + PACK * NT],
                    mybir.ActivationFunctionType.Exp,
                    bias=bias_sb[:LC], scale=1.0,
                )
                pe_sbs[p] = pe

            def pack_mm2(p):
                pe = pe_sbs.pop(p)
                for i in range(PACK):
                    c = p * PACK + i
                    nc.tensor.matmul(
                        arena[:EA, UT0:UT0 + NT],
                        xa_sb[:, c, :],
                        pe[:LC, i * NT:(i + 1) * NT],
                        start=(c == 0), stop=(c == NLC - 1),
                    )

            # software-pipelined emission: the in-order PE never has a
            # ready mm1 queued behind an exp-waiting mm2, and pack mm2 is
            # deferred ~2 l-chunks so the pack exp is long done
            pending = []
            mm1(0)
            for c in range(NLC):
                act(c)
                if c + 1 < NLC:
                    mm1(c + 1)
                mm2(c)
                if c % PACK == PACK - 1:
                    p = c // PACK
                    pack_mm1(p)
                    pack_act(p)
                    pending.append(p)
                while pending and (c >= PACK * pending[0] + 5 or c == NLC - 1):
                    pack_mm2(pending.pop(0))

            # U^T [EA, NSP] psum -> sbuf -> DRAM; divide/transpose on host
            u_sb = u_pool.tile([EA, NSP], F32, tag="u", name=f"u{b}")
            nc.vector.tensor_copy(u_sb[:, 0:NMAIN], arena[:EA, U0:U0 + NMAIN])
            nc.vector.tensor_copy(u_sb[:, NMAIN:NSP], arena[:EA, UT0:UT0 + NT])
            nc.sync.dma_start(out=m_d[b], in_=u_sb[:])
    nc.compile()
    return nc


def _get_nc():
    if not _NC:
        _NC.append(_build())
    return _NC[0]


def kernel(x, label_feature):
    global LAST_RESULT
    x = np.ascontiguousarray(np.asarray(x, dtype=np.float32))
    lf = np.ascontiguousarray(np.asarray(label_feature, dtype=np.float32))
    assert x.shape == (B, L, E) and lf.shape == (N_TOTAL, E)

    xa = np.zeros((B, LP, EA), ml_dtypes.bfloat16)
    xa[:, :L, :E] = x.astype(ml_dtypes.bfloat16)
    xa[:, :L, E] = 1.0
    xt = np.zeros((B, E, LP), np.float16)
    xt[:, :, :L] = x.transpose(0, 2, 1).astype(np.float16)

    in_maps = []
    for r in range(NCORES):
        lo = r * NS
        hi = min(lo + NS, N_TOTAL)
        labT = np.zeros((E, NSP), np.float16)
        labT[:, : hi - lo] = lf[lo:hi].T.astype(np.float16)
        in_maps.append({"xt": xt, "xa": xa, "labT": labT})

    nc = _get_nc()
    res = run_bass_kernel_spmd(
        nc, in_maps, core_ids=list(range(NCORES)), trace=TRACE
    )
    LAST_RESULT = res

    out = np.empty((B, N_TOTAL, E), np.float32)
    for r in range(NCORES):
        lo = r * NS
        hi = min(lo + NS, N_TOTAL)
        u = res.results[r]["m"]                      # [B, EA, NSP] f32
        m = u[:, :E, : hi - lo] / u[:, E:EA, : hi - lo]
        out[:, lo:hi, :] = m.transpose(0, 2, 1)
    return out


# revision 9
# speedup vs baseline: 3.1100x; 1.0217x over previous
"""CodeWiseAttention kernel for Trainium2 (8 NeuronCores, label-dim sharded).

m[b,n,:] = softmax(label_feature[n] @ x[b].T) @ x[b]

Sharding: label rows N=8922 split across 8 cores (1116/core, padded to 1152);
x replicated.  Per core, per batch b:
  mm1 (fp16):  S^T[l,n] = xT[e,l].T @ labT[e,n]     (xT, labT pre-transposed
               on host; fp16 runs the PE at 1 cycle/row vs ~3.5 for fp32)
  exp on ScalarE: expS = exp(S - 30) -> bf16        (constant shift; cancels)
  mm2 (bf16):  Uaug^T[e',n] += xa[l,e'].T @ expS^T[l,n]  accumulated over l,
               where xa has a ones column so row 100 of Uaug = Z = sum_l expS.
  out: DMA Uaug^T [101, n] to DRAM; host divides by Z and transposes.

PSUM layout (8 banks x 512 fp32 per partition), chosen so every matmul
output region sits inside one bank, accumulator banks are never touched by
score writes (start=True clears has_written for the WHOLE bank), and no
bank is PE-written while ScalarE/VectorE reads it (fatal collision; the
Tile tracker would serialize):
  b0 [0:512]      scores buf A, n[0:512]     } one contiguous 1024-wide
  b1 [512:1024]   scores buf A, n[512:1024]  }   ACTIVATE per l-chunk
  b2 [1024:1536]  scores buf B, n[0:512]
  b3 [1536:2048]  scores buf B, n[512:1024]
  b4 [2048:2560]  U accum, n[0:512]
  b5 [2560:3072]  U accum, n[512:1024]
  b6 [3072:3584]  "pack" scores: n[1024:1152] for 4 l-chunks at once
  b7 [3584:3712]  U accum, n[1024:1152]
The 128-wide n[1024:1152] tail is handled in 5 packs of 4 l-chunks so its
exp also runs as a few large ACTIVATEs (~293ns fixed cost per ACTIVATE).
"""
import numpy as np
import ml_dtypes
from contextlib import ExitStack

import concourse.tile as tile
from concourse import bacc, mybir
from concourse.bass_utils import run_bass_kernel_spmd

F32 = mybir.dt.float32
F16 = mybir.dt.float16
BF16 = mybir.dt.bfloat16

B, L, E = 8, 2500, 100
LP = 2520          # L padded; pad rows have xa=0 (incl. ones col) so they
                   # contribute nothing to U or Z even though exp(0-30) != 0
N_TOTAL = 8922
NCORES = 8
NS = 1116          # label rows per core (core 7: 1110 real)
NSP = 1152         # padded per-core label rows
NMAIN = 1024       # n columns handled by the main (512,512) loop
NT = 128           # tail n columns handled by 4-packs
LC = 126           # l-chunk rows (partition dim of S^T)
NLC = LP // LC     # 20 l-chunks
PACK = 4           # l-chunks per tail pack
NPACK = NLC // PACK
EA = E + 1         # x augmented with ones column
EXP_BIAS = -30.0

SA0, SB0 = 0, 1024     # main score buffer offsets (f32 elems)
U0 = 2048              # main U accumulator offset
PS0 = 3072             # pack score offset
UT0 = 3584             # tail U accumulator offset

TRACE = False
LAST_RESULT = None

_NC = []


def _build():
    nc = bacc.Bacc("TRN2", target_bir_lowering=False, debug=False)
    xt_d = nc.dram_tensor("xt", [B, E, LP], F16, kind="ExternalInput").ap()
    # xa pre-rearranged on host to [B, LC, NLC, EA] so the DMA is contiguous
    xa_d = nc.dram_tensor("xa", [B, LC, NLC, EA], BF16,
                          kind="ExternalInput").ap()
    lab_d = nc.dram_tensor("labT", [E, NSP], F16, kind="ExternalInput").ap()
    m_d = nc.dram_tensor("m", [B, EA, NSP], F32, kind="ExternalOutput").ap()

    with tile.TileContext(nc) as tc, ExitStack() as ctx:
        consts = ctx.enter_context(tc.tile_pool(name="consts", bufs=1))
        xt_pool = ctx.enter_context(tc.tile_pool(name="xtp", bufs=2))
        xa_pool = ctx.enter_context(tc.tile_pool(name="xap", bufs=2))
        e_pool = ctx.enter_context(tc.tile_pool(name="ep", bufs=3))
        et_pool = ctx.enter_context(tc.tile_pool(name="etp", bufs=2))
        u_pool = ctx.enter_context(tc.tile_pool(name="up", bufs=2))
        ps = ctx.enter_context(tc.tile_pool(name="ps", bufs=1, space="PSUM"))

        arena = ps.tile([128, 4096], F32)

        labT = consts.tile([E, NSP], F16)
        nc.sync.dma_start(out=labT[:], in_=lab_d)
        bias_sb = consts.tile([128, 1], F32)
        nc.vector.memset(bias_sb[:], EXP_BIAS)

        xt_tiles, xa_tiles = {}, {}

        def fetch(b):
            xt_tiles[b] = xt_pool.tile([E, LP], F16, tag="xt", name=f"xt{b}")
            nc.sync.dma_start(out=xt_tiles[b][:], in_=xt_d[b])
            xa_tiles[b] = xa_pool.tile(
                [LC, NLC, EA], BF16, tag="xa", name=f"xa{b}")
            nc.sync.dma_start(out=xa_tiles[b][:], in_=xa_d[b])

        fetch(0)
        prev_tail = [None]
        for b in range(B):
            xT = xt_tiles.pop(b)
            xa_sb = xa_tiles.pop(b)
            if b + 1 < B:
                fetch(b + 1)

            e_sbs, pe_sbs = {}, {}

            def mm1(c):
                base = SA0 if c % 2 == 0 else SB0
                for j in range(2):
                    mi = nc.tensor.matmul(
                        arena[:LC, base + j * 512:base + (j + 1) * 512],
                        xT[:, c * LC:(c + 1) * LC],
                        labT[:, j * 512:(j + 1) * 512],
                    )
                    if j == 1:
                        # same stationary weights as j==0: skip the reload
                        mi.ins.ldweights = False

            def act(c):
                base = SA0 if c % 2 == 0 else SB0
                e_sb = e_pool.tile([128, NMAIN], BF16, tag="e", name=f"e{c}")
                nc.scalar.activation(
                    e_sb[:LC, :], arena[:LC, base:base + NMAIN],
                    mybir.ActivationFunctionType.Exp,
                    bias=bias_sb[:LC], scale=1.0,
                )
                e_sbs[c] = e_sb

            def mm2(c):
                e_sb = e_sbs.pop(c)
                for j in range(2):
                    mi = nc.tensor.matmul(
                        arena[:EA, U0 + j * 512:U0 + (j + 1) * 512],
                        xa_sb[:, c, :],
                        e_sb[:LC, j * 512:(j + 1) * 512],
                        start=(c == 0), stop=(c == NLC - 1),
                    )
                    if j == 1:
                        mi.ins.ldweights = False

            def pack_mm1(p):
                for i in range(PACK):
                    c = p * PACK + i
                    nc.tensor.matmul(
                        arena[:LC, PS0 + i * NT:PS0 + (i + 1) * NT],
                        xT[:, c * LC:(c + 1) * LC],
                        labT[:, NMAIN:NSP],
                    )

            def pack_act(p):
                pe = et_pool.tile([128, PACK * NT], BF16, tag="pe",
                                  name=f"pe{p}")
                nc.scalar.activation(
                    pe[:LC, :], arena[:LC, PS0:PS0 + PACK * NT],
                    mybir.ActivationFunctionType.Exp,
                    bias=bias_sb[:LC], scale=1.0,
                )
                pe_sbs[p] = pe

            def pack_mm2(p):
                pe = pe_sbs.pop(p)
                for i in range(PACK):
                    c = p * PACK + i
                    nc.tensor.matmul(
                        arena[:EA, UT0:UT0 + NT],
                        xa_sb[:, c, :],
                        pe[:LC, i * NT:(i + 1) * NT],
                        start=(c == 0), stop=(c == NLC - 1),
                    )

            # software-pipelined emission: the in-order PE never has a
            # ready mm1 queued behind an exp-waiting mm2, and pack mm2 is
            # deferred ~2 l-chunks so the pack exp is long done.  The LAST
            # pack's mm2 (plus the U-tail copy and the output DMA) is
            # deferred into the NEXT batch so the PE doesn't stall at the
            # batch boundary waiting for the final pack exp.
            pending = []
            mm1(0)
            for c in range(NLC):
                act(c)
                if c + 1 < NLC:
                    mm1(c + 1)
                mm2(c)
                if c == 0 and prev_tail[0] is not None:
                    prev_tail[0]()
                    prev_tail[0] = None
                if c % PACK == PACK - 1:
                    p = c // PACK
                    pack_mm1(p)
                    pack_act(p)
                    pending.append(p)
                while pending and c >= PACK * pending[0] + 5:
                    pack_mm2(pending.pop(0))

            # U^T [EA, NSP] psum -> sbuf -> DRAM; divide/transpose on host.
            # Main part copied now (so next batch's mm2 can reuse its banks);
            # tail part deferred with the last pack.
            u_sb = u_pool.tile([EA, NSP], F32, tag="u", name=f"u{b}")
            nc.vector.tensor_copy(u_sb[:, 0:NMAIN], arena[:EA, U0:U0 + NMAIN])
            p_last = pending.pop()
            assert not pending

            # bind the batch-b tiles eagerly: xa_sb/pe_sbs are rebound
            # every batch iteration, and tail runs during batch b+1
            def tail(p=p_last, u=u_sb, b=b, xa_cur=xa_sb,
                     pe=pe_sbs.pop(p_last)):
                for i in range(PACK):
                    c = p * PACK + i
                    nc.tensor.matmul(
                        arena[:EA, UT0:UT0 + NT],
                        xa_cur[:, c, :],
                        pe[:LC, i * NT:(i + 1) * NT],
                        start=(c == 0), stop=(c == NLC - 1),
                    )
                nc.vector.tensor_copy(u[:, NMAIN:NSP], arena[:EA, UT0:UT0 + NT])
                nc.sync.dma_start(out=m_d[b], in_=u[:])

            prev_tail[0] = tail
        prev_tail[0]()
    nc.compile()
    return nc


def _get_nc():
    if not _NC:
        _NC.append(_build())
    return _NC[0]


def kernel(x, label_feature):
    global LAST_RESULT
    x = np.ascontiguousarray(np.asarray(x, dtype=np.float32))
    lf = np.ascontiguousarray(np.asarray(label_feature, dtype=np.float32))
    assert x.shape == (B, L, E) and lf.shape == (N_TOTAL, E)

    xa_f = np.zeros((B, LP, EA), np.float32)
    xa_f[:, :L, :E] = x
    xa_f[:, :L, E] = 1.0
    # [B, LP, EA] -> [B, LC, NLC, EA] so the device DMA is contiguous
    xa = np.ascontiguousarray(
        xa_f.reshape(B, NLC, LC, EA).transpose(0, 2, 1, 3)
    ).astype(ml_dtypes.bfloat16)
    xt = np.zeros((B, E, LP), np.float16)
    xt[:, :, :L] = x.transpose(0, 2, 1).astype(np.float16)

    in_maps = []
    for r in range(NCORES):
        lo = r * NS
        hi = min(lo + NS, N_TOTAL)
        labT = np.zeros((E, NSP), np.float16)
        labT[:, : hi - lo] = lf[lo:hi].T.astype(np.float16)
        in_maps.append({"xt": xt, "xa": xa, "labT": labT})

    nc = _get_nc()
    res = run_bass_kernel_spmd(
        nc, in_maps, core_ids=list(range(NCORES)), trace=TRACE
    )
    LAST_RESULT = res

    out = np.empty((B, N_TOTAL, E), np.float32)
    for r in range(NCORES):
        lo = r * NS
        hi = min(lo + NS, N_TOTAL)
        u = res.results[r]["m"]                      # [B, EA, NSP] f32
        m = u[:, :E, : hi - lo] / u[:, E:EA, : hi - lo]
        out[:, lo:hi, :] = m.transpose(0, 2, 1)
    return out
